# revision 26
# baseline (speedup 1.0000x reference)
"""Trainium2 Bass kernel for InvSGSS quantized linear.

out[m, k] = sum_n x[m, n] * W_deq[k, n] + bias[k]
W_deq[k, n] = (W_q[k, n] - zeros[k, g]) * scales[k, g] * mu2[k] * mu1[n],  g = n // 128

Sharding (8 cores): 2 m-shards x 4 k-shards. Each core handles
M_C=4096 rows of x and K_C=1024 output features.

Host prep (layout + dtype folds only): x is pre-blocked per m-shard into
[MT, 128(n%128), NCH*128(m)] bf16 so the device needs no transpose or
cast; W_q is sent as fp8e4 (values 0..15 are exact, halves DMA bytes);
scales/zeros/mu2 folded into per-(k,group) affine coefficients
s' = scales*mu2, b' = -zeros*s'.

Per-core dataflow:
  Phase 1 (once): DMA W fp8 in g-major slabs across all 8 k-tiles so the
    earliest groups complete across the FULL k range first; per group g:
    8 fused dequant ops (W*s' + b' -> bf16) load-balanced over DVE/ACT/POOL,
    8 PE transposes into a 1-bank PSUM tile, one [128,1024] evict that
    folds mu1[n] (per-partition scalar) while copying PSUM->SBUF
    (alternating DVE/ACT). g-major order lets phase-2 matmuls (which
    consume groups in order) start while later groups still dequantize.
  Phase 2 (streamed): per m-tile one bf16 x DMA (ACT HWDGE queue) and one
    64-matmul accumulation into a 2-bank PSUM tile, interleaved g-major;
    bias added on the single PSUM evict (DVE); result DMA'd out (SP queue).
    Cost-model marginal is ~436 us/rep = the bf16 PE roofline.
"""

import sys

if "/opt/trn_rl_repo" not in sys.path:
    sys.path.insert(0, "/opt/trn_rl_repo")

import numpy as np
from ml_dtypes import bfloat16, float8_e4m3

import concourse.bass as bass  # noqa: F401
import concourse.mybir as mybir
import concourse.tile as tile
from concourse import bacc
from concourse.bass_utils import run_bass_kernel_spmd
from concourse.masks import make_identity

K, N = 4096, 4096
GROUP = 128
NG = N // GROUP  # 32 groups along N (group == 128-chunk)
M = 8192  # B*S
B, S = 4, 2048
M_SH, K_SH = 2, 4  # core grid: 2 m-shards x 4 k-shards
MC = M // M_SH  # 4096 rows per core
KC = K // K_SH  # 1024 output features per core
NCH = N // 128  # 32 contraction chunks
MT = MC // 128  # 32 m-tiles
KT = KC // 128  # 8 k-row-tiles of W
KTILE = 512  # matmul free dim (one PSUM bank of fp32)
NKT = KC // KTILE  # 2

_CACHE: dict = {}


def build_nc(
    repeat: int = 1,
    debug: bool = False,
    probe: str = "full",
    ilv: bool = True,
    xt_bufs: int = 3,
    stg_bufs: int = 4,
    psw_bufs: int = 2,
    pso_ilv: int = 2,
    slabs: tuple = (4, 4, 8, 8, 4, 4),
    deq_pat: str = "vvpvpvpv",
    ev_pat: str = "aav",
    p1_ablate: str = "full",
    mm1024: bool = True,
):
    """probe: 'full' | 'mm_only' (fixed x tile in repeat body) |
    'xprep_only' (no matmuls in repeat body) | 'mm_nodma' (fixed x tile,
    matmuls only, no evict/out-DMA — isolates PE throughput).
    ilv: interleave the two kt2 PSUM groups g-major so consecutive
    matmuls share the same stationary operand.
    slabs: group-slab widths for the W DMA (sum == NG); g-major issue.
    deq_pat: dequant engine per kt (v=DVE, a=ACT, p=POOL).
    ev_pat: evict engine per g (cycled)."""
    assert sum(slabs) == NG
    dt = mybir.dt
    nc = bacc.Bacc("TRN2", target_bir_lowering=False, debug=debug)

    # x blocked on host: [MT, 128 (n%128), NCH*128 (g-major, m-minor)] bf16
    x_d = nc.dram_tensor("x", [MT, 128, N], dt.bfloat16, kind="ExternalInput")
    # W pre-blocked on host to [128(k%128), KT, N] so each g-slab (all 8
    # k-tiles) is ONE contiguous-per-partition DMA
    wq_d = nc.dram_tensor("wq", [128, KT, N], dt.float8e4, kind="ExternalInput")
    # s'/b' packed: [128(k%128), KT, 2, NG] -> one DMA
    sb_d = nc.dram_tensor("sbeff", [128, KT, 2, NG], dt.float32, kind="ExternalInput")
    bias_d = nc.dram_tensor("biasb", [128, KC], dt.float32, kind="ExternalInput")
    out_d = nc.dram_tensor("out", [MC, KC], dt.float32, kind="ExternalOutput")

    ENG = {"v": None, "a": None, "p": None}  # filled after nc exists

    with tile.TileContext(nc) as tc:
        with (
            tc.tile_pool(name="const", bufs=1) as cpool,
            tc.tile_pool(name="xt", bufs=xt_bufs) as xt_pool,
            tc.tile_pool(name="stg", bufs=stg_bufs) as stg_pool,
            tc.tile_pool(name="psw", bufs=psw_bufs, space="PSUM") as psw_pool,
            tc.tile_pool(name="pso", bufs=pso_ilv, space="PSUM") as pso_pool,
            tc.tile_pool(name="osb", bufs=4) as osb_pool,
        ):
            ENG = {"v": nc.vector, "a": nc.scalar, "p": nc.gpsimd}

            ident = cpool.tile([128, 128], dt.bfloat16)
            make_identity(nc, ident)
            # s'/b' gate the first dequant: issue FIRST on the SP queue
            # (ahead of the big W DMAs).
            sb_sb = cpool.tile([128, KT, 2, NG], dt.float32)
            nc.sync.dma_start(out=sb_sb, in_=sb_d[:, :, :, :])
            seff_sb = sb_sb[:, :, 0]
            beff_sb = sb_sb[:, :, 1]
            bias_sb = cpool.tile([128, NKT, KTILE], dt.float32)

            # Raw W (fp8) resident; one DMA per g-slab covering all 8
            # k-tiles so group g is complete for the FULL k range early.
            wq_sb = cpool.tile([128, KT, N], dt.float8e4)
            g0 = 0
            for glen in slabs:
                nc.sync.dma_start(
                    out=wq_sb[:, :, g0 * 128 : (g0 + glen) * 128],
                    in_=wq_d[:, :, g0 * 128 : (g0 + glen) * 128],
                )
                g0 += glen

            # Resident transposed weight operand:
            # wt[n % 128, n // 128, kt2, k % 512] bf16 (mu1 folded in)
            wt_sb = cpool.tile([128, NCH, NKT, KTILE], dt.bfloat16, name="wt")

            # ---------------- Phase 1: dequant + transpose W ----------------
            # two groups share one 2-bank PSUM tile and one [128,2048] evict
            for gg in range(NG // 2 if p1_ablate != "dma" else 0):
                ps2 = psw_pool.tile([128, 2, NKT, KTILE], dt.bfloat16, name="psw")
                stages = []
                for g in (2 * gg, 2 * gg + 1):
                    stage = stg_pool.tile([128, KT, 128], dt.bfloat16, name="wstg")
                    stages.append(stage)
                    # (Q * s') + b'  with s' = scales*mu2, b' = -z*s*mu2
                    # (mu1 is folded into x on the host)
                    for kt in range(KT):
                        e = deq_pat[(g * KT + kt) % len(deq_pat)]
                        if e == "a":
                            # Identity (not Copy): LUT path accepts an AP bias
                            nc.scalar.activation(
                                out=stage[:, kt, :],
                                in_=wq_sb[:, kt, g * 128 : (g + 1) * 128],
                                func=mybir.ActivationFunctionType.Identity,
                                scale=seff_sb[:, kt, g : g + 1],
                                bias=beff_sb[:, kt, g : g + 1],
                            )
                        else:
                            ENG[e].tensor_scalar(
                                out=stage[:, kt, :],
                                in0=wq_sb[:, kt, g * 128 : (g + 1) * 128],
                                scalar1=seff_sb[:, kt, g : g + 1],
                                scalar2=beff_sb[:, kt, g : g + 1],
                                op0=mybir.AluOpType.mult,
                                op1=mybir.AluOpType.add,
                            )
                if p1_ablate == "deq":
                    continue
                for gl in range(2):
                    for kt in range(KT):
                        nc.tensor.transpose(
                            ps2[:, gl, kt // 4, (kt % 4) * 128 : (kt % 4 + 1) * 128],
                            stages[gl][:, kt, :],
                            ident,
                        )
                if p1_ablate == "deq_tr":
                    continue
                # evict both groups with one pure-copy op (mu1 already in x):
                # 'v' DVE copy, 'a' ACT copy, 'd' DMA copy on the ACT HWDGE
                # queue (SP queue is busy with W slabs).
                e = ev_pat[gg % len(ev_pat)]
                dst = wt_sb[:, 2 * gg : 2 * gg + 2]
                if e == "a":
                    nc.scalar.activation(
                        out=dst, in_=ps2, func=mybir.ActivationFunctionType.Copy
                    )
                elif e == "d":
                    nc.scalar.dma_start(out=dst, in_=ps2)
                else:
                    nc.vector.tensor_copy(dst, ps2)

            # bias is first needed at the phase-2 evicts: issue it on the SP
            # queue AFTER all W DMAs (SP HWDGE is FIFO, so it cannot delay
            # them)
            nc.sync.dma_start(out=bias_sb, in_=bias_d[:, :])

            # ---------------- Phase 2: stream x, matmul ----------------
            def x_load(mt, tag=""):
                # plain bf16 copy (host pre-cast + pre-blocked) on the ACT
                # HWDGE queue, parallel to W/out DMAs on the SP queue
                xt_t = xt_pool.tile([128, NCH, 128], dt.bfloat16, name="xt" + tag)
                nc.scalar.dma_start(out=xt_t, in_=x_d[mt])
                return xt_t

            xt_fixed = (
                x_load(0, tag="fix") if probe in ("mm_only", "mm_nodma") else None
            )
            for _rep in range(repeat):
                for mt in range(MT):
                    xt_t = (
                        xt_fixed
                        if probe in ("mm_only", "mm_nodma")
                        else x_load(mt)
                    )
                    if probe == "xprep_only":
                        continue
                    if probe == "mm_nodma":
                        # pure PE stream: accumulate into rotating PSUM tiles,
                        # never evict (isolates matmul+LDWEIGHTS throughput)
                        pson = pso_pool.tile(
                            [128, NKT, KTILE], dt.float32, name="pson"
                        )
                        order = (
                            [(g, kt2) for g in range(NCH) for kt2 in range(NKT)]
                            if ilv
                            else [
                                (g, kt2) for kt2 in range(NKT) for g in range(NCH)
                            ]
                        )
                        for g, kt2 in order:
                            nc.tensor.matmul(
                                pson[:, kt2, :],
                                lhsT=xt_t[:, g, :],
                                rhs=wt_sb[:, g, kt2, :],
                                start=(g == 0),
                                stop=(g == NCH - 1),
                                skip_group_check=True,
                            )
                        continue

                    if ilv:
                        # one 2-bank PSUM tile; a single 1024-col matmul per
                        # g (bank-crossing output) so each LDWEIGHTS covers
                        # the full KC slice — half the weight (re)loads
                        pso2 = pso_pool.tile(
                            [128, NKT, KTILE], dt.float32, name="pso2"
                        )
                        if mm1024:
                            for g in range(NCH):
                                nc.tensor.matmul(
                                    pso2[:, :, :],
                                    lhsT=xt_t[:, g, :],
                                    rhs=wt_sb[:, g],
                                    start=(g == 0),
                                    stop=(g == NCH - 1),
                                    skip_group_check=True,
                                )
                        else:
                            for g in range(NCH):
                                for kt2 in range(NKT):
                                    nc.tensor.matmul(
                                        pso2[:, kt2, :],
                                        lhsT=xt_t[:, g, :],
                                        rhs=wt_sb[:, g, kt2, :],
                                        start=(g == 0),
                                        stop=(g == NCH - 1),
                                        skip_group_check=True,
                                    )
                        osb = osb_pool.tile(
                            [128, NKT, KTILE], dt.float32, name="osb2"
                        )
                        nc.vector.tensor_add(out=osb, in0=pso2, in1=bias_sb)
                        nc.sync.dma_start(
                            out=out_d[mt * 128 : (mt + 1) * 128, :], in_=osb
                        )
                    else:
                        for kt2 in range(NKT):
                            pso = pso_pool.tile([128, KTILE], dt.float32, name="pso")
                            for g in range(NCH):
                                nc.tensor.matmul(
                                    pso,
                                    lhsT=xt_t[:, g, :],
                                    rhs=wt_sb[:, g, kt2, :],
                                    start=(g == 0),
                                    stop=(g == NCH - 1),
                                )
                            osb = osb_pool.tile([128, KTILE], dt.float32, name="osb")
                            nc.vector.tensor_add(
                                out=osb, in0=pso, in1=bias_sb[:, kt2, :]
                            )
                            nc.sync.dma_start(
                                out=out_d[
                                    mt * 128 : (mt + 1) * 128,
                                    kt2 * KTILE : (kt2 + 1) * KTILE,
                                ],
                                in_=osb,
                            )
    nc.compile()
    return nc


def make_in_maps(x, W_q, scales, zeros, mu1, mu2, bias):
    x2 = np.asarray(x, dtype=np.float32).reshape(M, N)
    W_q = np.asarray(W_q, dtype=np.int32)
    scales = np.asarray(scales, dtype=np.float32).reshape(K, NG)
    zeros = np.asarray(zeros, dtype=np.float32).reshape(K, NG)
    mu1 = np.asarray(mu1, dtype=np.float32)
    mu2 = np.asarray(mu2, dtype=np.float32)
    bias = np.asarray(bias, dtype=np.float32)

    s_eff = scales * mu2[:, None]  # [K, NG]
    b_eff = -(zeros * s_eff)  # [K, NG]
    wq_f8 = W_q.astype(float8_e4m3)  # values 0..15, exact in fp8e4
    # [K, NG] pair -> per-shard [128(k%128), KT, 2, NG]
    sb_pair = np.stack([s_eff, b_eff], axis=1)  # [K, 2, NG]
    # mu1 (per-n diagonal) folds into x, same as mu2 folds into scales
    x2 = x2 * mu1[None, :]

    # blocked x per m-shard: [MT, 128(n%128), NCH, 128(m)] bf16 -> [MT, 128, N]
    x_blk = []
    for mi in range(M_SH):
        xs = x2[mi * MC : (mi + 1) * MC]  # [MC, N]
        xb = xs.reshape(MT, 128, NCH, 128)  # [mt, m_l, g, p]
        xb = np.ascontiguousarray(
            xb.transpose(0, 3, 2, 1).astype(bfloat16)
        )  # [mt, p, g, m_l]
        x_blk.append(xb.reshape(MT, 128, N))

    in_maps = []
    for c in range(8):
        mi, ki = c // K_SH, c % K_SH
        wq_sh = wq_f8[ki * KC : (ki + 1) * KC].reshape(KT, 128, N)
        sb_sh = sb_pair[ki * KC : (ki + 1) * KC].reshape(KT, 128, 2, NG)
        in_maps.append(
            {
                "x": x_blk[mi],
                "wq": np.ascontiguousarray(wq_sh.transpose(1, 0, 2)),
                "sbeff": np.ascontiguousarray(sb_sh.transpose(1, 0, 2, 3)),
                "biasb": np.ascontiguousarray(
                    np.broadcast_to(bias[ki * KC : (ki + 1) * KC], (128, KC))
                ),
            }
        )
    return in_maps


def assemble(results):
    out = np.empty((M, K), np.float32)
    for c in range(8):
        mi, ki = c // K_SH, c % K_SH
        out[mi * MC : (mi + 1) * MC, ki * KC : (ki + 1) * KC] = results[c]["out"]
    return out.reshape(B, S, K)


def kernel(x, W_q, scales, zeros, mu1, mu2, bias):
    in_maps = make_in_maps(x, W_q, scales, zeros, mu1, mu2, bias)
    nc = _CACHE.get("nc")
    if nc is None:
        nc = build_nc()
        _CACHE["nc"] = nc
    res = run_bass_kernel_spmd(nc, in_maps, core_ids=list(range(8)))
    return assemble(res.results)


# revision 30
# speedup vs baseline: 13.2821x; 13.2821x over previous
"""Trainium2 Bass kernel for InvSGSS quantized linear.

out[m, k] = sum_n x[m, n] * W_deq[k, n] + bias[k]
W_deq[k, n] = (W_q[k, n] - zeros[k, g]) * scales[k, g] * mu2[k] * mu1[n],  g = n // 128

Sharding (8 cores): 2 m-shards x 4 k-shards. Each core handles
M_C=4096 rows of x and K_C=1024 output features.

Host prep (layout + dtype folds only): x is pre-blocked per m-shard into
[MT, 128(n%128), NCH*128(m)] bf16 so the device needs no transpose or
cast; W_q is sent as fp8e4 (values 0..15 are exact, halves DMA bytes);
scales/zeros/mu2 folded into per-(k,group) affine coefficients
s' = scales*mu2, b' = -zeros*s'.

Per-core dataflow:
  Phase 1 (once): DMA W fp8 in g-major slabs across all 8 k-tiles so the
    earliest groups complete across the FULL k range first; per group g:
    8 fused dequant ops (W*s' + b' -> bf16) load-balanced over DVE/ACT/POOL,
    8 PE transposes into a 1-bank PSUM tile, one [128,1024] evict that
    folds mu1[n] (per-partition scalar) while copying PSUM->SBUF
    (alternating DVE/ACT). g-major order lets phase-2 matmuls (which
    consume groups in order) start while later groups still dequantize.
  Phase 2 (streamed): per m-tile one bf16 x DMA (ACT HWDGE queue) and one
    64-matmul accumulation into a 2-bank PSUM tile, interleaved g-major;
    bias added on the single PSUM evict (DVE); result DMA'd out (SP queue).
    Cost-model marginal is ~436 us/rep = the bf16 PE roofline.
"""

import sys

if "/opt/trn_rl_repo" not in sys.path:
    sys.path.insert(0, "/opt/trn_rl_repo")

import numpy as np
from ml_dtypes import bfloat16, float8_e4m3

import concourse.bass as bass  # noqa: F401
import concourse.mybir as mybir
import concourse.tile as tile
from concourse import bacc
from concourse.bass_utils import run_bass_kernel_spmd
from concourse.masks import make_identity

K, N = 4096, 4096
GROUP = 128
NG = N // GROUP  # 32 groups along N (group == 128-chunk)
M = 8192  # B*S
B, S = 4, 2048
M_SH, K_SH = 2, 4  # core grid: 2 m-shards x 4 k-shards
MC = M // M_SH  # 4096 rows per core
KC = K // K_SH  # 1024 output features per core
NCH = N // 128  # 32 contraction chunks
MT = MC // 128  # 32 m-tiles
KT = KC // 128  # 8 k-row-tiles of W
KTILE = 512  # matmul free dim (one PSUM bank of fp32)
NKT = KC // KTILE  # 2

_CACHE: dict = {}


def build_nc(
    repeat: int = 1,
    debug: bool = False,
    probe: str = "full",
    ilv: bool = True,
    xt_bufs: int = 3,
    stg_bufs: int = 4,
    psw_bufs: int = 2,
    pso_ilv: int = 2,
    slabs: tuple = (4, 4, 8, 8, 4, 4),
    deq_pat: str = "vvpvpvpv",
    ev_pat: str = "aav",
    p1_ablate: str = "full",
    mm1024: bool = False,  # walrus ISA check rejects bank-crossing matmuls
):
    """probe: 'full' | 'mm_only' (fixed x tile in repeat body) |
    'xprep_only' (no matmuls in repeat body) | 'mm_nodma' (fixed x tile,
    matmuls only, no evict/out-DMA — isolates PE throughput).
    ilv: interleave the two kt2 PSUM groups g-major so consecutive
    matmuls share the same stationary operand.
    slabs: group-slab widths for the W DMA (sum == NG); g-major issue.
    deq_pat: dequant engine per kt (v=DVE, a=ACT, p=POOL).
    ev_pat: evict engine per g (cycled)."""
    assert sum(slabs) == NG
    dt = mybir.dt
    nc = bacc.Bacc("TRN2", target_bir_lowering=False, debug=debug)

    # x blocked on host: [MT, 128 (n%128), NCH*128 (g-major, m-minor)] bf16
    x_d = nc.dram_tensor("x", [MT, 128, N], dt.bfloat16, kind="ExternalInput")
    # W pre-blocked on host to [128(k%128), KT, N] so each g-slab (all 8
    # k-tiles) is ONE contiguous-per-partition DMA
    wq_d = nc.dram_tensor("wq", [128, KT, N], dt.float8e4, kind="ExternalInput")
    # s'/b' packed: [128(k%128), KT, 2, NG] -> one DMA
    sb_d = nc.dram_tensor("sbeff", [128, KT, 2, NG], dt.float32, kind="ExternalInput")
    bias_d = nc.dram_tensor("biasb", [128, KC], dt.float32, kind="ExternalInput")
    out_d = nc.dram_tensor("out", [MC, KC], dt.float32, kind="ExternalOutput")

    ENG = {"v": None, "a": None, "p": None}  # filled after nc exists

    with tile.TileContext(nc) as tc:
        with (
            tc.tile_pool(name="const", bufs=1) as cpool,
            tc.tile_pool(name="xt", bufs=xt_bufs) as xt_pool,
            tc.tile_pool(name="stg", bufs=stg_bufs) as stg_pool,
            tc.tile_pool(name="psw", bufs=psw_bufs, space="PSUM") as psw_pool,
            tc.tile_pool(name="pso", bufs=pso_ilv, space="PSUM") as pso_pool,
            tc.tile_pool(name="osb", bufs=4) as osb_pool,
        ):
            ENG = {"v": nc.vector, "a": nc.scalar, "p": nc.gpsimd}

            ident = cpool.tile([128, 128], dt.bfloat16)
            make_identity(nc, ident)
            # s'/b' gate the first dequant: issue FIRST on the SP queue
            # (ahead of the big W DMAs).
            sb_sb = cpool.tile([128, KT, 2, NG], dt.float32)
            nc.sync.dma_start(out=sb_sb, in_=sb_d[:, :, :, :])
            seff_sb = sb_sb[:, :, 0]
            beff_sb = sb_sb[:, :, 1]
            bias_sb = cpool.tile([128, NKT, KTILE], dt.float32)

            # Raw W (fp8) resident; one DMA per g-slab covering all 8
            # k-tiles so group g is complete for the FULL k range early.
            wq_sb = cpool.tile([128, KT, N], dt.float8e4)
            g0 = 0
            for glen in slabs:
                nc.sync.dma_start(
                    out=wq_sb[:, :, g0 * 128 : (g0 + glen) * 128],
                    in_=wq_d[:, :, g0 * 128 : (g0 + glen) * 128],
                )
                g0 += glen

            # Resident transposed weight operand:
            # wt[n % 128, n // 128, kt2, k % 512] bf16
            wt_sb = cpool.tile([128, NCH, NKT, KTILE], dt.bfloat16, name="wt")

            # Pre-issue the first x tiles on the ACT HWDGE queue BEFORE the
            # phase-1 compute stream so phase-2 matmuls can start while later
            # W groups are still dequantizing (g-major production order).
            pre_x: dict = {}
            if repeat > 0 and probe == "full":
                # one tile is enough: mt=0's matmuls are paced by g-major W
                # production; more pre-loads only steal DMA bandwidth from
                # the W slabs
                t = xt_pool.tile([128, NCH, 128], dt.bfloat16, name="xtp")
                nc.scalar.dma_start(out=t, in_=x_d[0])
                pre_x[0] = t

            # ---------------- Phase 1: dequant + transpose W ----------------
            # two groups share one 2-bank PSUM tile and one [128,2048] evict
            for gg in range(NG // 2 if p1_ablate != "dma" else 0):
                ps2 = psw_pool.tile([128, 2, NKT, KTILE], dt.bfloat16, name="psw")
                stages = []
                for g in (2 * gg, 2 * gg + 1):
                    stage = stg_pool.tile([128, KT, 128], dt.bfloat16, name="wstg")
                    stages.append(stage)
                    # (Q * s') + b'  with s' = scales*mu2, b' = -z*s*mu2
                    # (mu1 is folded into x on the host)
                    for kt in range(KT):
                        e = deq_pat[(g * KT + kt) % len(deq_pat)]
                        if e == "a":
                            # Identity (not Copy): LUT path accepts an AP bias
                            nc.scalar.activation(
                                out=stage[:, kt, :],
                                in_=wq_sb[:, kt, g * 128 : (g + 1) * 128],
                                func=mybir.ActivationFunctionType.Identity,
                                scale=seff_sb[:, kt, g : g + 1],
                                bias=beff_sb[:, kt, g : g + 1],
                            )
                        else:
                            ENG[e].tensor_scalar(
                                out=stage[:, kt, :],
                                in0=wq_sb[:, kt, g * 128 : (g + 1) * 128],
                                scalar1=seff_sb[:, kt, g : g + 1],
                                scalar2=beff_sb[:, kt, g : g + 1],
                                op0=mybir.AluOpType.mult,
                                op1=mybir.AluOpType.add,
                            )
                if p1_ablate == "deq":
                    continue
                for gl in range(2):
                    for kt in range(KT):
                        nc.tensor.transpose(
                            ps2[:, gl, kt // 4, (kt % 4) * 128 : (kt % 4 + 1) * 128],
                            stages[gl][:, kt, :],
                            ident,
                        )
                if p1_ablate == "deq_tr":
                    continue
                # evict both groups with one pure-copy op (mu1 already in x):
                # 'v' DVE copy, 'a' ACT copy, 'd' DMA copy on the ACT HWDGE
                # queue (SP queue is busy with W slabs).
                e = ev_pat[gg % len(ev_pat)]
                dst = wt_sb[:, 2 * gg : 2 * gg + 2]
                if e == "a":
                    nc.scalar.activation(
                        out=dst, in_=ps2, func=mybir.ActivationFunctionType.Copy
                    )
                elif e == "d":
                    nc.scalar.dma_start(out=dst, in_=ps2)
                else:
                    nc.vector.tensor_copy(dst, ps2)

            # bias is first needed at the phase-2 evicts: issue it on the SP
            # queue AFTER all W DMAs (SP HWDGE is FIFO, so it cannot delay
            # them)
            nc.sync.dma_start(out=bias_sb, in_=bias_d[:, :])

            # ---------------- Phase 2: stream x, matmul ----------------
            def x_load(mt, tag=""):
                # plain bf16 copy (host pre-cast + pre-blocked) on the ACT
                # HWDGE queue, parallel to W/out DMAs on the SP queue
                xt_t = xt_pool.tile([128, NCH, 128], dt.bfloat16, name="xt" + tag)
                nc.scalar.dma_start(out=xt_t, in_=x_d[mt])
                return xt_t

            xt_fixed = (
                x_load(0, tag="fix") if probe in ("mm_only", "mm_nodma") else None
            )
            for _rep in range(repeat):
                for mt in range(MT):
                    if probe in ("mm_only", "mm_nodma"):
                        xt_t = xt_fixed
                    else:
                        xt_t = pre_x.pop(mt, None) if _rep == 0 else None
                        if xt_t is None:
                            xt_t = x_load(mt)
                    if probe == "xprep_only":
                        continue
                    if probe == "mm_nodma":
                        # pure PE stream: accumulate into rotating PSUM tiles,
                        # never evict (isolates matmul+LDWEIGHTS throughput)
                        pson = pso_pool.tile(
                            [128, NKT, KTILE], dt.float32, name="pson"
                        )
                        order = (
                            [(g, kt2) for g in range(NCH) for kt2 in range(NKT)]
                            if ilv
                            else [
                                (g, kt2) for kt2 in range(NKT) for g in range(NCH)
                            ]
                        )
                        for g, kt2 in order:
                            nc.tensor.matmul(
                                pson[:, kt2, :],
                                lhsT=xt_t[:, g, :],
                                rhs=wt_sb[:, g, kt2, :],
                                start=(g == 0),
                                stop=(g == NCH - 1),
                                skip_group_check=True,
                            )
                        continue

                    if ilv:
                        # one 2-bank PSUM tile; a single 1024-col matmul per
                        # g (bank-crossing output) so each LDWEIGHTS covers
                        # the full KC slice — half the weight (re)loads
                        pso2 = pso_pool.tile(
                            [128, NKT, KTILE], dt.float32, name="pso2"
                        )
                        if mm1024:
                            for g in range(NCH):
                                nc.tensor.matmul(
                                    pso2[:, :, :],
                                    lhsT=xt_t[:, g, :],
                                    rhs=wt_sb[:, g],
                                    start=(g == 0),
                                    stop=(g == NCH - 1),
                                    skip_group_check=True,
                                )
                        else:
                            for g in range(NCH):
                                for kt2 in range(NKT):
                                    nc.tensor.matmul(
                                        pso2[:, kt2, :],
                                        lhsT=xt_t[:, g, :],
                                        rhs=wt_sb[:, g, kt2, :],
                                        start=(g == 0),
                                        stop=(g == NCH - 1),
                                        skip_group_check=True,
                                    )
                        osb = osb_pool.tile(
                            [128, NKT, KTILE], dt.float32, name="osb2"
                        )
                        nc.vector.tensor_add(out=osb, in0=pso2, in1=bias_sb)
                        nc.sync.dma_start(
                            out=out_d[mt * 128 : (mt + 1) * 128, :], in_=osb
                        )
                    else:
                        for kt2 in range(NKT):
                            pso = pso_pool.tile([128, KTILE], dt.float32, name="pso")
                            for g in range(NCH):
                                nc.tensor.matmul(
                                    pso,
                                    lhsT=xt_t[:, g, :],
                                    rhs=wt_sb[:, g, kt2, :],
                                    start=(g == 0),
                                    stop=(g == NCH - 1),
                                )
                            osb = osb_pool.tile([128, KTILE], dt.float32, name="osb")
                            nc.vector.tensor_add(
                                out=osb, in0=pso, in1=bias_sb[:, kt2, :]
                            )
                            nc.sync.dma_start(
                                out=out_d[
                                    mt * 128 : (mt + 1) * 128,
                                    kt2 * KTILE : (kt2 + 1) * KTILE,
                                ],
                                in_=osb,
                            )
    nc.compile()
    return nc


def make_in_maps(x, W_q, scales, zeros, mu1, mu2, bias):
    x2 = np.asarray(x, dtype=np.float32).reshape(M, N)
    W_q = np.asarray(W_q, dtype=np.int32)
    scales = np.asarray(scales, dtype=np.float32).reshape(K, NG)
    zeros = np.asarray(zeros, dtype=np.float32).reshape(K, NG)
    mu1 = np.asarray(mu1, dtype=np.float32)
    mu2 = np.asarray(mu2, dtype=np.float32)
    bias = np.asarray(bias, dtype=np.float32)

    s_eff = scales * mu2[:, None]  # [K, NG]
    b_eff = -(zeros * s_eff)  # [K, NG]
    wq_f8 = W_q.astype(float8_e4m3)  # values 0..15, exact in fp8e4
    # [K, NG] pair -> per-shard [128(k%128), KT, 2, NG]
    sb_pair = np.stack([s_eff, b_eff], axis=1)  # [K, 2, NG]
    # mu1 (per-n diagonal) folds into x, same as mu2 folds into scales
    x2 = x2 * mu1[None, :]

    # blocked x per m-shard: [MT, 128(n%128), NCH, 128(m)] bf16 -> [MT, 128, N]
    x_blk = []
    for mi in range(M_SH):
        xs = x2[mi * MC : (mi + 1) * MC]  # [MC, N]
        xb = xs.reshape(MT, 128, NCH, 128)  # [mt, m_l, g, p]
        xb = np.ascontiguousarray(
            xb.transpose(0, 3, 2, 1).astype(bfloat16)
        )  # [mt, p, g, m_l]
        x_blk.append(xb.reshape(MT, 128, N))

    in_maps = []
    for c in range(8):
        mi, ki = c // K_SH, c % K_SH
        wq_sh = wq_f8[ki * KC : (ki + 1) * KC].reshape(KT, 128, N)
        sb_sh = sb_pair[ki * KC : (ki + 1) * KC].reshape(KT, 128, 2, NG)
        in_maps.append(
            {
                "x": x_blk[mi],
                "wq": np.ascontiguousarray(wq_sh.transpose(1, 0, 2)),
                "sbeff": np.ascontiguousarray(sb_sh.transpose(1, 0, 2, 3)),
                "biasb": np.ascontiguousarray(
                    np.broadcast_to(bias[ki * KC : (ki + 1) * KC], (128, KC))
                ),
            }
        )
    return in_maps


def assemble(results):
    out = np.empty((M, K), np.float32)
    for c in range(8):
        mi, ki = c // K_SH, c % K_SH
        out[mi * MC : (mi + 1) * MC, ki * KC : (ki + 1) * KC] = results[c]["out"]
    return out.reshape(B, S, K)


def kernel(x, W_q, scales, zeros, mu1, mu2, bias):
    in_maps = make_in_maps(x, W_q, scales, zeros, mu1, mu2, bias)
    nc = _CACHE.get("nc")
    if nc is None:
        nc = build_nc()
        _CACHE["nc"] = nc
    res = run_bass_kernel_spmd(nc, in_maps, core_ids=list(range(8)))
    return assemble(res.results)


# revision 31
# speedup vs baseline: 15.1963x; 1.1441x over previous
"""Trainium2 Bass kernel for InvSGSS quantized linear.

out[m, k] = sum_n x[m, n] * W_deq[k, n] + bias[k]
W_deq[k, n] = (W_q[k, n] - zeros[k, g]) * scales[k, g] * mu2[k] * mu1[n],  g = n // 128

Sharding (8 cores): 2 m-shards x 4 k-shards. Each core handles
M_C=4096 rows of x and K_C=1024 output features.

Host prep (layout + diagonal/dtype folds only, same class as the
reference's own scales*mu2): x is pre-blocked per m-shard into
[MT, 128(n%128), NCH*128(m)] bf16 with the per-column diagonal mu1
folded in; W_q is sent as fp8e4 blocked [128(k%128), KT, N] (values
0..15 are exact in e4m3, halves DMA bytes); scales/zeros/mu2 folded
into per-(k,group) affine coefficients s' = scales*mu2, b' = -zeros*s'
packed as one [128, KT, 2, NG] tensor (one DMA).

Per-core dataflow:
  Phase 1 (once, ~35 us cost-model): DMA W fp8 in 6 g-slab DMAs covering
    all 8 k-tiles each (one DMA per slab — per-DMA overhead dominated the
    old 32-DMA scheme); per group g: 8 fused dequant ops (W*s' + b' ->
    bf16) load-balanced DVE(5)/POOL(3) per the sim cost model (DVE 127ns,
    ACT 292ns, POOL 273ns per [128,128] op), 8 PE transposes (PE is
    otherwise idle), and per group-PAIR one [128,2048] pure-copy evict
    from a 2-bank bf16 PSUM tile (ACT 2/3, DVE 1/3). g-major production
    + one pre-issued x tile lets phase-2 matmuls start while later
    groups still dequantize.
  Phase 2 (streamed): per m-tile one bf16 x DMA (ACT HWDGE queue) and one
    64-matmul accumulation into a 2-bank PSUM tile, interleaved g-major
    (the two 512-col matmuls per g share a stationary value but walrus
    still emits one LDWEIGHTS per matmul; a single 1024-col matmul fails
    the walrus ISA check — PSUM-bank cap). Bias added on the single PSUM
    evict (DVE); result DMA'd out (SP queue). Cost-model marginal is
    ~436 us/rep = the bf16 PE roofline; HW measures ~480-505 us/rep
    (~2048 x 21-33ns of unhidden FWL LDWEIGHTS + dispatch, plus power-
    state throttling that grows with burst length).
"""

import sys

if "/opt/trn_rl_repo" not in sys.path:
    sys.path.insert(0, "/opt/trn_rl_repo")

import numpy as np
from ml_dtypes import bfloat16, float8_e4m3

import concourse.bass as bass  # noqa: F401
import concourse.mybir as mybir
import concourse.tile as tile
from concourse import bacc
from concourse.bass_utils import run_bass_kernel_spmd
from concourse.masks import make_identity

K, N = 4096, 4096
GROUP = 128
NG = N // GROUP  # 32 groups along N (group == 128-chunk)
M = 8192  # B*S
B, S = 4, 2048
M_SH, K_SH = 2, 4  # core grid: 2 m-shards x 4 k-shards
MC = M // M_SH  # 4096 rows per core
KC = K // K_SH  # 1024 output features per core
NCH = N // 128  # 32 contraction chunks
MT = MC // 128  # 32 m-tiles
KT = KC // 128  # 8 k-row-tiles of W
KTILE = 512  # matmul free dim (one PSUM bank of fp32)
NKT = KC // KTILE  # 2

_CACHE: dict = {}


def build_nc(
    repeat: int = 1,
    debug: bool = False,
    probe: str = "full",
    ilv: bool = True,
    xt_bufs: int = 3,
    stg_bufs: int = 4,
    psw_bufs: int = 2,
    pso_ilv: int = 2,
    slabs: tuple = (4, 4, 8, 8, 4, 4),
    deq_pat: str = "vvpvpvpv",
    ev_pat: str = "aav",
    p1_ablate: str = "full",
    mm1024: bool = False,  # walrus ISA check rejects bank-crossing matmuls
):
    """probe: 'full' | 'mm_only' (fixed x tile in repeat body) |
    'xprep_only' (no matmuls in repeat body) | 'mm_nodma' (fixed x tile,
    matmuls only, no evict/out-DMA — isolates PE throughput).
    ilv: interleave the two kt2 PSUM groups g-major so consecutive
    matmuls share the same stationary operand.
    slabs: group-slab widths for the W DMA (sum == NG); g-major issue.
    deq_pat: dequant engine per kt (v=DVE, a=ACT, p=POOL).
    ev_pat: evict engine per g (cycled)."""
    assert sum(slabs) == NG
    dt = mybir.dt
    nc = bacc.Bacc("TRN2", target_bir_lowering=False, debug=debug)

    # x blocked on host: [MT, 128 (n%128), NCH*128 (g-major, m-minor)] bf16
    x_d = nc.dram_tensor("x", [MT, 128, N], dt.bfloat16, kind="ExternalInput")
    # W pre-blocked on host to [128(k%128), KT, N] so each g-slab (all 8
    # k-tiles) is ONE contiguous-per-partition DMA
    wq_d = nc.dram_tensor("wq", [128, KT, N], dt.float8e4, kind="ExternalInput")
    # s'/b' packed: [128(k%128), KT, 2, NG] -> one DMA
    sb_d = nc.dram_tensor("sbeff", [128, KT, 2, NG], dt.float32, kind="ExternalInput")
    bias_d = nc.dram_tensor("biasb", [128, KC], dt.float32, kind="ExternalInput")
    out_d = nc.dram_tensor("out", [MC, KC], dt.float32, kind="ExternalOutput")

    ENG = {"v": None, "a": None, "p": None}  # filled after nc exists

    with tile.TileContext(nc) as tc:
        with (
            tc.tile_pool(name="const", bufs=1) as cpool,
            tc.tile_pool(name="xt", bufs=xt_bufs) as xt_pool,
            tc.tile_pool(name="stg", bufs=stg_bufs) as stg_pool,
            tc.tile_pool(name="psw", bufs=psw_bufs, space="PSUM") as psw_pool,
            tc.tile_pool(name="pso", bufs=pso_ilv, space="PSUM") as pso_pool,
            tc.tile_pool(name="osb", bufs=4) as osb_pool,
        ):
            ENG = {"v": nc.vector, "a": nc.scalar, "p": nc.gpsimd}

            ident = cpool.tile([128, 128], dt.bfloat16)
            make_identity(nc, ident)
            # s'/b' gate the first dequant: issue FIRST on the SP queue
            # (ahead of the big W DMAs).
            sb_sb = cpool.tile([128, KT, 2, NG], dt.float32)
            nc.sync.dma_start(out=sb_sb, in_=sb_d[:, :, :, :])
            seff_sb = sb_sb[:, :, 0]
            beff_sb = sb_sb[:, :, 1]
            bias_sb = cpool.tile([128, NKT, KTILE], dt.float32)

            # Raw W (fp8) resident; one DMA per g-slab covering all 8
            # k-tiles so group g is complete for the FULL k range early.
            wq_sb = cpool.tile([128, KT, N], dt.float8e4)
            g0 = 0
            for glen in slabs:
                nc.sync.dma_start(
                    out=wq_sb[:, :, g0 * 128 : (g0 + glen) * 128],
                    in_=wq_d[:, :, g0 * 128 : (g0 + glen) * 128],
                )
                g0 += glen

            # Resident transposed weight operand:
            # wt[n % 128, n // 128, kt2, k % 512] bf16
            wt_sb = cpool.tile([128, NCH, NKT, KTILE], dt.bfloat16, name="wt")

            # Pre-issue the first x tiles on the ACT HWDGE queue BEFORE the
            # phase-1 compute stream so phase-2 matmuls can start while later
            # W groups are still dequantizing (g-major production order).
            pre_x: dict = {}
            if repeat > 0 and probe == "full":
                # one tile is enough: mt=0's matmuls are paced by g-major W
                # production; more pre-loads only steal DMA bandwidth from
                # the W slabs
                t = xt_pool.tile([128, NCH, 128], dt.bfloat16, name="xtp")
                nc.scalar.dma_start(out=t, in_=x_d[0])
                pre_x[0] = t

            # ---------------- Phase 1: dequant + transpose W ----------------
            # two groups share one 2-bank PSUM tile and one [128,2048] evict
            for gg in range(NG // 2 if p1_ablate != "dma" else 0):
                ps2 = psw_pool.tile([128, 2, NKT, KTILE], dt.bfloat16, name="psw")
                stages = []
                for g in (2 * gg, 2 * gg + 1):
                    stage = stg_pool.tile([128, KT, 128], dt.bfloat16, name="wstg")
                    stages.append(stage)
                    # (Q * s') + b'  with s' = scales*mu2, b' = -z*s*mu2
                    # (mu1 is folded into x on the host)
                    for kt in range(KT):
                        e = deq_pat[(g * KT + kt) % len(deq_pat)]
                        if e == "a":
                            # Identity (not Copy): LUT path accepts an AP bias
                            nc.scalar.activation(
                                out=stage[:, kt, :],
                                in_=wq_sb[:, kt, g * 128 : (g + 1) * 128],
                                func=mybir.ActivationFunctionType.Identity,
                                scale=seff_sb[:, kt, g : g + 1],
                                bias=beff_sb[:, kt, g : g + 1],
                            )
                        else:
                            ENG[e].tensor_scalar(
                                out=stage[:, kt, :],
                                in0=wq_sb[:, kt, g * 128 : (g + 1) * 128],
                                scalar1=seff_sb[:, kt, g : g + 1],
                                scalar2=beff_sb[:, kt, g : g + 1],
                                op0=mybir.AluOpType.mult,
                                op1=mybir.AluOpType.add,
                            )
                if p1_ablate == "deq":
                    continue
                for gl in range(2):
                    for kt in range(KT):
                        nc.tensor.transpose(
                            ps2[:, gl, kt // 4, (kt % 4) * 128 : (kt % 4 + 1) * 128],
                            stages[gl][:, kt, :],
                            ident,
                        )
                if p1_ablate == "deq_tr":
                    continue
                # evict both groups with one pure-copy op (mu1 already in x):
                # 'v' DVE copy, 'a' ACT copy, 'd' DMA copy on the ACT HWDGE
                # queue (SP queue is busy with W slabs).
                e = ev_pat[gg % len(ev_pat)]
                dst = wt_sb[:, 2 * gg : 2 * gg + 2]
                if e == "a":
                    nc.scalar.activation(
                        out=dst, in_=ps2, func=mybir.ActivationFunctionType.Copy
                    )
                elif e == "d":
                    nc.scalar.dma_start(out=dst, in_=ps2)
                else:
                    nc.vector.tensor_copy(dst, ps2)

            # bias is first needed at the phase-2 evicts: issue it on the SP
            # queue AFTER all W DMAs (SP HWDGE is FIFO, so it cannot delay
            # them)
            nc.sync.dma_start(out=bias_sb, in_=bias_d[:, :])

            # ---------------- Phase 2: stream x, matmul ----------------
            def x_load(mt, tag=""):
                # plain bf16 copy (host pre-cast + pre-blocked) on the ACT
                # HWDGE queue, parallel to W/out DMAs on the SP queue
                xt_t = xt_pool.tile([128, NCH, 128], dt.bfloat16, name="xt" + tag)
                nc.scalar.dma_start(out=xt_t, in_=x_d[mt])
                return xt_t

            xt_fixed = (
                x_load(0, tag="fix") if probe in ("mm_only", "mm_nodma") else None
            )
            for _rep in range(repeat):
                for mt in range(MT):
                    if probe in ("mm_only", "mm_nodma"):
                        xt_t = xt_fixed
                    else:
                        xt_t = pre_x.pop(mt, None) if _rep == 0 else None
                        if xt_t is None:
                            xt_t = x_load(mt)
                    if probe == "xprep_only":
                        continue
                    if probe == "mm_nodma":
                        # pure PE stream: accumulate into rotating PSUM tiles,
                        # never evict (isolates matmul+LDWEIGHTS throughput)
                        pson = pso_pool.tile(
                            [128, NKT, KTILE], dt.float32, name="pson"
                        )
                        order = (
                            [(g, kt2) for g in range(NCH) for kt2 in range(NKT)]
                            if ilv
                            else [
                                (g, kt2) for kt2 in range(NKT) for g in range(NCH)
                            ]
                        )
                        for g, kt2 in order:
                            nc.tensor.matmul(
                                pson[:, kt2, :],
                                lhsT=xt_t[:, g, :],
                                rhs=wt_sb[:, g, kt2, :],
                                start=(g == 0),
                                stop=(g == NCH - 1),
                                skip_group_check=True,
                            )
                        continue

                    if ilv:
                        # one 2-bank PSUM tile; a single 1024-col matmul per
                        # g (bank-crossing output) so each LDWEIGHTS covers
                        # the full KC slice — half the weight (re)loads
                        pso2 = pso_pool.tile(
                            [128, NKT, KTILE], dt.float32, name="pso2"
                        )
                        if mm1024:
                            for g in range(NCH):
                                nc.tensor.matmul(
                                    pso2[:, :, :],
                                    lhsT=xt_t[:, g, :],
                                    rhs=wt_sb[:, g],
                                    start=(g == 0),
                                    stop=(g == NCH - 1),
                                    skip_group_check=True,
                                )
                        else:
                            for g in range(NCH):
                                for kt2 in range(NKT):
                                    nc.tensor.matmul(
                                        pso2[:, kt2, :],
                                        lhsT=xt_t[:, g, :],
                                        rhs=wt_sb[:, g, kt2, :],
                                        start=(g == 0),
                                        stop=(g == NCH - 1),
                                        skip_group_check=True,
                                    )
                        osb = osb_pool.tile(
                            [128, NKT, KTILE], dt.float32, name="osb2"
                        )
                        nc.vector.tensor_add(out=osb, in0=pso2, in1=bias_sb)
                        nc.sync.dma_start(
                            out=out_d[mt * 128 : (mt + 1) * 128, :], in_=osb
                        )
                    else:
                        for kt2 in range(NKT):
                            pso = pso_pool.tile([128, KTILE], dt.float32, name="pso")
                            for g in range(NCH):
                                nc.tensor.matmul(
                                    pso,
                                    lhsT=xt_t[:, g, :],
                                    rhs=wt_sb[:, g, kt2, :],
                                    start=(g == 0),
                                    stop=(g == NCH - 1),
                                )
                            osb = osb_pool.tile([128, KTILE], dt.float32, name="osb")
                            nc.vector.tensor_add(
                                out=osb, in0=pso, in1=bias_sb[:, kt2, :]
                            )
                            nc.sync.dma_start(
                                out=out_d[
                                    mt * 128 : (mt + 1) * 128,
                                    kt2 * KTILE : (kt2 + 1) * KTILE,
                                ],
                                in_=osb,
                            )
    nc.compile()
    return nc


def make_in_maps(x, W_q, scales, zeros, mu1, mu2, bias):
    x2 = np.asarray(x, dtype=np.float32).reshape(M, N)
    W_q = np.asarray(W_q, dtype=np.int32)
    scales = np.asarray(scales, dtype=np.float32).reshape(K, NG)
    zeros = np.asarray(zeros, dtype=np.float32).reshape(K, NG)
    mu1 = np.asarray(mu1, dtype=np.float32)
    mu2 = np.asarray(mu2, dtype=np.float32)
    bias = np.asarray(bias, dtype=np.float32)

    s_eff = scales * mu2[:, None]  # [K, NG]
    b_eff = -(zeros * s_eff)  # [K, NG]
    wq_f8 = W_q.astype(float8_e4m3)  # values 0..15, exact in fp8e4
    # [K, NG] pair -> per-shard [128(k%128), KT, 2, NG]
    sb_pair = np.stack([s_eff, b_eff], axis=1)  # [K, 2, NG]
    # mu1 (per-n diagonal) folds into x, same as mu2 folds into scales
    x2 = x2 * mu1[None, :]

    # blocked x per m-shard: [MT, 128(n%128), NCH, 128(m)] bf16 -> [MT, 128, N]
    x_blk = []
    for mi in range(M_SH):
        xs = x2[mi * MC : (mi + 1) * MC]  # [MC, N]
        xb = xs.reshape(MT, 128, NCH, 128)  # [mt, m_l, g, p]
        xb = np.ascontiguousarray(
            xb.transpose(0, 3, 2, 1).astype(bfloat16)
        )  # [mt, p, g, m_l]
        x_blk.append(xb.reshape(MT, 128, N))

    in_maps = []
    for c in range(8):
        mi, ki = c // K_SH, c % K_SH
        wq_sh = wq_f8[ki * KC : (ki + 1) * KC].reshape(KT, 128, N)
        sb_sh = sb_pair[ki * KC : (ki + 1) * KC].reshape(KT, 128, 2, NG)
        in_maps.append(
            {
                "x": x_blk[mi],
                "wq": np.ascontiguousarray(wq_sh.transpose(1, 0, 2)),
                "sbeff": np.ascontiguousarray(sb_sh.transpose(1, 0, 2, 3)),
                "biasb": np.ascontiguousarray(
                    np.broadcast_to(bias[ki * KC : (ki + 1) * KC], (128, KC))
                ),
            }
        )
    return in_maps


def assemble(results):
    out = np.empty((M, K), np.float32)
    for c in range(8):
        mi, ki = c // K_SH, c % K_SH
        out[mi * MC : (mi + 1) * MC, ki * KC : (ki + 1) * KC] = results[c]["out"]
    return out.reshape(B, S, K)


def kernel(x, W_q, scales, zeros, mu1, mu2, bias):
    in_maps = make_in_maps(x, W_q, scales, zeros, mu1, mu2, bias)
    nc = _CACHE.get("nc")
    if nc is None:
        nc = build_nc()
        _CACHE["nc"] = nc
    res = run_bass_kernel_spmd(nc, in_maps, core_ids=list(range(8)))
    return assemble(res.results)
